# revision 1
# baseline (speedup 1.0000x reference)
"""Trainium2 Bass kernel for the contrastive loss:

    epos = exp(cos_sim(q_pos, img_pos))   # [2B] rows, D=1024
    eneg = exp(cos_sim(q_neg, img_neg))   # [23B]
    pos_sum = segsum(epos, 2); neg_sum = segsum(eneg, 23)   # [B]
    loss = sum((neg_sum - pos_sum) / (pos_sum + neg_sum + 0.001))

Data-parallel over 8 NeuronCores: core c takes batch items [c*512, (c+1)*512),
i.e. rows [c*1024,(c+1)*1024) of the pos tensors and [c*11776,(c+1)*11776) of
the neg tensors. Each core emits its 512 per-item values; the host sums.

Per-core layout: local item i = 4*p + s (partition p in [0,128), slot s in
[0,4)), so partition p owns pos rows 8p..8p+7 and neg rows 92p..92p+91 of the
core's shard — each partition's rows are contiguous in DRAM, so every DMA is
128 partitions x (4 rows * 4KiB) contiguous.

Per 128-row slice [128, 1024]: the row-wise dot runs on the vector engine as
one fused scalar_tensor_tensor ((a*1)*b with accum_out), and the two
sum-of-squares run on the scalar engine as Square activations with accum_out.
A fraction of the b-squares is moved to the vector engine to balance the two
engines; both stay below the ~290us/core DMA floor (100 MiB @ ~358 GB/s).
"""

import numpy as np

import concourse.bass as bass
import concourse.tile as tile
from concourse import mybir
from concourse.bass_utils import run_bass_kernel_spmd

EPS_COS = 1e-8
EP = 0.001

N_CORES = 8
P = 128            # SBUF partitions
D = 1024           # embedding dim
B_FULL = 4096      # total batch items
ITEMS = B_FULL // N_CORES   # 512 items per core
SLOTS = ITEMS // P          # 4 items per partition
J_POS = SLOTS * 2           # 8 pos rows per partition
J_NEG = SLOTS * 23          # 92 neg rows per partition
G = 4                       # j-slices per DMA chunk (2 MiB per tensor)

F32 = mybir.dt.float32
ALU = mybir.AluOpType
ACTF = mybir.ActivationFunctionType


def _split_multiwait_instructions(nc):
    """The walrus build here rejects >1 sync-wait per instruction; hoist extra
    waits onto single-wait NOPs placed just before the instruction."""
    ctr = 0
    for fn in nc.m.functions:
        for bb in fn.blocks:
            insts = list(bb.instructions)
            if not any(
                i.sync_info is not None and len(i.sync_info.on_wait) > 1
                for i in insts
            ):
                continue
            new_insts = []
            for inst in insts:
                si = inst.sync_info
                if si is not None and len(si.on_wait) > 1:
                    waits = list(si.on_wait)
                    is_drain = type(inst).__name__ == "InstDrain"
                    keep = [] if is_drain else waits[-1:]
                    move = waits if is_drain else waits[:-1]
                    for w in move:
                        ctr += 1
                        new_insts.append(
                            mybir.InstNoOp(
                                name=f"I-wsplit-{ctr}",
                                engine=inst.engine,
                                sync_info=mybir.SyncInfo(on_wait=[w], on_update=[]),
                                text_hint="wsplit",
                            )
                        )
                    si.on_wait = keep
                new_insts.append(inst)
            bb.instructions = new_insts


def _emit_slice_ops(nc, a_t, b_t, g, dots, na2s, nb2s, j, scr_v, scr_s, sq_b_on_dve):
    """Stats for one [128, 1024] row-slice: dot(a,b), sum(a^2), sum(b^2)."""
    a_sl = a_t[:, g, :]
    b_sl = b_t[:, g, :]
    nc.vector.scalar_tensor_tensor(
        out=scr_v[:],
        in0=a_sl,
        scalar=1.0,
        in1=b_sl,
        op0=ALU.mult,
        op1=ALU.mult,
        accum_out=dots[:, j : j + 1],
    )
    nc.scalar.activation(
        out=scr_s[:],
        in_=a_sl,
        func=ACTF.Square,
        accum_out=na2s[:, j : j + 1],
    )
    if sq_b_on_dve:
        nc.vector.scalar_tensor_tensor(
            out=scr_v[:],
            in0=b_sl,
            scalar=1.0,
            in1=b_sl,
            op0=ALU.mult,
            op1=ALU.mult,
            accum_out=nb2s[:, j : j + 1],
        )
    else:
        nc.scalar.activation(
            out=scr_s[:],
            in_=b_sl,
            func=ACTF.Square,
            accum_out=nb2s[:, j : j + 1],
        )


def _emit_exp_cos(nc, pool, dots, na2s, nb2s, n, e_out):
    """e_out[:, :n] = exp(dot / max(sqrt(na2*nb2), 1e-8)) (elementwise)."""
    prod = pool.tile([P, n], F32, tag=f"prod{n}")
    nc.vector.tensor_tensor(
        out=prod[:], in0=na2s[:], in1=nb2s[:], op=ALU.mult
    )
    nc.vector.tensor_scalar(
        out=prod[:], in0=prod[:], scalar1=EPS_COS * EPS_COS, scalar2=None,
        op0=ALU.max,
    )
    # prod <- sqrt(prod) = max(na*nb, eps)
    nc.scalar.activation(out=prod[:], in_=prod[:], func=ACTF.Sqrt)
    rec = pool.tile([P, n], F32, tag=f"rec{n}")
    nc.vector.reciprocal(out=rec[:], in_=prod[:])
    cos = pool.tile([P, n], F32, tag=f"cos{n}")
    nc.vector.tensor_tensor(out=cos[:], in0=dots[:], in1=rec[:], op=ALU.mult)
    nc.scalar.activation(out=e_out[:], in_=cos[:], func=ACTF.Exp)


def build_bass():
    nc = bass.Bass()
    qp = nc.declare_dram_parameter("qp", [P * J_POS, D], F32, isOutput=False)
    pi = nc.declare_dram_parameter("pi", [P * J_POS, D], F32, isOutput=False)
    qn = nc.declare_dram_parameter("qn", [P * J_NEG, D], F32, isOutput=False)
    ni = nc.declare_dram_parameter("ni", [P * J_NEG, D], F32, isOutput=False)
    out = nc.declare_dram_parameter("out", [P, SLOTS], F32, isOutput=True)

    qp_v = qp[:].rearrange("(p j) d -> p j d", j=J_POS)
    pi_v = pi[:].rearrange("(p j) d -> p j d", j=J_POS)
    qn_v = qn[:].rearrange("(p j) d -> p j d", j=J_NEG)
    ni_v = ni[:].rearrange("(p j) d -> p j d", j=J_NEG)

    with tile.TileContext(nc) as tc:
        with (
            tc.tile_pool(name="io", bufs=3) as io,
            tc.tile_pool(name="st", bufs=1) as st,
            tc.tile_pool(name="tail", bufs=1) as tail,
        ):
            dot_p = st.tile([P, J_POS], F32)
            na2_p = st.tile([P, J_POS], F32)
            nb2_p = st.tile([P, J_POS], F32)
            dot_n = st.tile([P, J_NEG], F32)
            na2_n = st.tile([P, J_NEG], F32)
            nb2_n = st.tile([P, J_NEG], F32)
            scr_v = st.tile([P, D], F32)
            scr_s = st.tile([P, D], F32)

            # (a_view, b_view, dots, na2, nb2, j0) per chunk of G slices
            chunks = []
            for c in range(J_POS // G):
                chunks.append((qp_v, pi_v, dot_p, na2_p, nb2_p, c * G))
            for c in range(J_NEG // G):
                chunks.append((qn_v, ni_v, dot_n, na2_n, nb2_n, c * G))

            slice_idx = 0
            for a_v, b_v, dots, na2s, nb2s, j0 in chunks:
                a_t = io.tile([P, G, D], F32, tag="a")
                b_t = io.tile([P, G, D], F32, tag="b")
                nc.sync.dma_start(out=a_t, in_=a_v[:, j0 : j0 + G, :])
                nc.sync.dma_start(out=b_t, in_=b_v[:, j0 : j0 + G, :])
                for g in range(G):
                    # ~3/8 of b-squares on the vector engine balances DVE/ACT
                    sq_b_on_dve = (slice_idx % 8) < 3
                    _emit_slice_ops(
                        nc, a_t, b_t, g, dots, na2s, nb2s, j0 + g,
                        scr_v, scr_s, sq_b_on_dve,
                    )
                    slice_idx += 1

            e_p = tail.tile([P, J_POS], F32)
            e_n = tail.tile([P, J_NEG], F32)
            _emit_exp_cos(nc, tail, dot_p, na2_p, nb2_p, J_POS, e_p)
            _emit_exp_cos(nc, tail, dot_n, na2_n, nb2_n, J_NEG, e_n)

            pos_sum = tail.tile([P, SLOTS], F32)
            neg_sum = tail.tile([P, SLOTS], F32)
            nc.vector.tensor_reduce(
                out=pos_sum[:],
                in_=e_p[:].rearrange("p (s t) -> p s t", t=2),
                axis=mybir.AxisListType.X,
                op=ALU.add,
            )
            nc.vector.tensor_reduce(
                out=neg_sum[:],
                in_=e_n[:].rearrange("p (s t) -> p s t", t=23),
                axis=mybir.AxisListType.X,
                op=ALU.add,
            )
            num = tail.tile([P, SLOTS], F32)
            den = tail.tile([P, SLOTS], F32)
            nc.vector.tensor_tensor(
                out=num[:], in0=neg_sum[:], in1=pos_sum[:], op=ALU.subtract
            )
            nc.vector.tensor_tensor(
                out=den[:], in0=neg_sum[:], in1=pos_sum[:], op=ALU.add
            )
            nc.vector.tensor_scalar(
                out=den[:], in0=den[:], scalar1=EP, scalar2=None, op0=ALU.add
            )
            rden = tail.tile([P, SLOTS], F32)
            nc.vector.reciprocal(out=rden[:], in_=den[:])
            per_item = tail.tile([P, SLOTS], F32)
            nc.vector.tensor_tensor(
                out=per_item[:], in0=num[:], in1=rden[:], op=ALU.mult
            )
            nc.sync.dma_start(out=out[:], in_=per_item[:])

    _split_multiwait_instructions(nc)
    return nc


_NC_CACHE = None


def _get_nc():
    global _NC_CACHE
    if _NC_CACHE is None:
        _NC_CACHE = build_bass()
    return _NC_CACHE


def kernel(question_embeddings_pos, question_embeddings_neg,
           pos_image_embeddings, neg_image_embeddings, batch_size=None,
           **_unused):
    qp = np.ascontiguousarray(np.asarray(question_embeddings_pos, dtype=np.float32))
    qn = np.ascontiguousarray(np.asarray(question_embeddings_neg, dtype=np.float32))
    pi = np.ascontiguousarray(np.asarray(pos_image_embeddings, dtype=np.float32))
    ni = np.ascontiguousarray(np.asarray(neg_image_embeddings, dtype=np.float32))

    rp = 2 * ITEMS   # pos rows per core
    rn = 23 * ITEMS  # neg rows per core
    in_maps = [
        {
            "qp": qp[c * rp : (c + 1) * rp],
            "pi": pi[c * rp : (c + 1) * rp],
            "qn": qn[c * rn : (c + 1) * rn],
            "ni": ni[c * rn : (c + 1) * rn],
        }
        for c in range(N_CORES)
    ]
    res = run_bass_kernel_spmd(_get_nc(), in_maps, list(range(N_CORES)))
    total = np.float64(0.0)
    for c in range(N_CORES):
        total += res.results[c]["out"].sum(dtype=np.float64)
    return np.float32(total)
